# revision 46
# baseline (speedup 1.0000x reference)
"""Multi-head self-attention (B=4, S=2048, D=1024, H=16) on 8 TRN2 NeuronCores.

Sharding: core c handles batch b = c // 2 and head-group g = c % 2
(8 heads, 512 hidden columns). Per core:
  - Q^T, K^T projections (d-major layout), V projection (token-major),
    emitted per 512-token group, need-ordered so the first score matmul
    fires a few microseconds in; later token groups are interleaved into
    the attention stream as PE filler (the scalar engine's exp stream is
    the global bottleneck).
  - Attention computed transposed: S^T[k, q] = K_h @ Q_h^T per 128-key
    block, two heads concurrently on disjoint PE row-groups; exp on the
    scalar engine (softmax max-subtraction is skipped — logits ~N(0,1)).
    Fully-masked query columns of partially-masked blocks are skipped in
    the scores matmul, exp, and AV accumulation; the remaining partial
    window is multiplied by a (1-mask) triangle on the vector engine.
  - O^T = V_aug^T @ P^T with a ones column appended to V so the softmax
    denominator drops out of the same matmul; normalize by its
    reciprocal.
  - Row-sharded output projection -> partial [512, 1024] bf16 per query
    group, one pairwise bf16 ReduceScatter per query group (the last
    group fires two half-sized ones so the collective overlaps its own
    projection), final rows DMA'd to the output.
Host reassembles: rows qg*512 + g*256 of batch b come from core (2*b+g);
the last query group is mapped per 128-row half-chunk.
"""

import sys

for _p in ("/opt/trn_rl_repo",):
    if _p not in sys.path:
        sys.path.insert(0, _p)

from contextlib import ExitStack

import ml_dtypes
import numpy as np

import concourse.bass as bass
from concourse import bacc
import concourse.mybir as mybir
import concourse.tile as tile
from concourse.bass_utils import run_bass_kernel_spmd

F32 = mybir.dt.float32
F32R = mybir.dt.float32r
BF16 = mybir.dt.bfloat16
MM_DT = BF16
AF = mybir.ActivationFunctionType
ALU = mybir.AluOpType

B, S, D, H, DEPTH = 4, 2048, 1024, 16, 64
HG = H // 2          # heads per core = 8
GD = HG * DEPTH      # local hidden width = 512
QG = 512             # query-group width (matmul N)
KB = 128             # key-block height (matmul M)
NQG = S // QG        # 4
NKB = S // KB        # 16
NCORES = 8

LAST_EXEC_NS = None
LAST_RESULTS = None

QG_ORDER = (1, 2, 3, 0)
GROUPS = [[0, 1], [2, 3], [4, 5], [6, 7]]


def _mask_schedule(mask2d):
    """Classify each (query-group, key-block) against the actual mask.

    Returns (sched, windows): sched[qg] is a list of (kb, c0, w0, w1, widx).
    Query columns [0, c0) of the block are fully masked and skipped
    entirely; columns [w0, w1) are partially masked and get multiplied by
    windows[widx] ([128, w1-w0], value 1-mask transposed to [k, q]);
    columns [w1, 512) are fully allowed. A block is dropped when fully
    masked. The first surviving block of each qg is forced to c0=0 so its
    AV/scores matmuls initialize the whole PSUM accumulation range.
    """
    wins = {}
    warr = []
    sched = []
    for qg in range(NQG):
        blocks = []
        for kb in range(NKB):
            blk = mask2d[qg * QG:(qg + 1) * QG, kb * KB:(kb + 1) * KB]  # [q, k]
            assert set(np.unique(blk)) <= {0.0, 1.0}, "non-binary mask"
            col_masked = blk.all(axis=1)      # [q] fully masked query?
            col_any = blk.any(axis=1)         # [q] any masked key?
            if col_masked.all():
                continue  # fully masked block
            c0 = 0
            while c0 < QG and col_masked[c0]:
                c0 += 1
            if not blocks:
                c0 = 0  # first block initializes full PSUM range
            w1 = QG
            while w1 > c0 and not col_any[w1 - 1]:
                w1 -= 1
            w0 = c0
            if w1 <= w0:
                blocks.append((kb, c0, 0, 0, None))
                continue
            w = (1.0 - np.ascontiguousarray(blk[w0:w1, :].T)).astype(np.float32)
            key = w.tobytes()
            if key not in wins:
                wins[key] = len(warr)
                warr.append(w)
            blocks.append((kb, c0, w0, w1, wins[key]))
        sched.append(blocks)
    assert len(warr) <= 16, "too many unique mask windows"
    return sched, warr


def _build(sched, windows):
    nc = bacc.Bacc(target_bir_lowering=False, trn_type="TRN2")

    xq = nc.dram_tensor("xq_t", [D, S], BF16, kind="ExternalInput")
    xk = nc.dram_tensor("xk_t", [D, S], BF16, kind="ExternalInput")
    xv = nc.dram_tensor("xv_t", [D, S], BF16, kind="ExternalInput")
    wq_d = nc.dram_tensor("wq_g", [D, GD], BF16, kind="ExternalInput")
    wk_d = nc.dram_tensor("wk_g", [D, GD], BF16, kind="ExternalInput")
    wv_d = nc.dram_tensor("wv_g", [D, GD], BF16, kind="ExternalInput")
    wo_d = nc.dram_tensor("wo_g", [GD, D], BF16, kind="ExternalInput")
    bq_d = nc.dram_tensor("bq_g", [GD], F32, kind="ExternalInput")
    bk_d = nc.dram_tensor("bk_g", [GD], F32, kind="ExternalInput")
    bv_d = nc.dram_tensor("bv_g", [GD], F32, kind="ExternalInput")
    bo_d = nc.dram_tensor("bo_h", [D], F32, kind="ExternalInput")
    out_d = nc.dram_tensor("out", [NQG * 256, D], BF16, kind="ExternalOutput")

    ones_d = nc.inline_tensor(np.ones((128, HG), dtype=ml_dtypes.bfloat16), "ones_col")
    onesf_d = nc.inline_tensor(np.ones((1, DEPTH), dtype=np.float32), "ones_row")
    nwin = len(windows)
    win_d = [
        nc.inline_tensor(w.astype(ml_dtypes.bfloat16), f"mwin{i}")
        for i, w in enumerate(windows)
    ]

    with tile.TileContext(nc) as tc, ExitStack() as ctx:
        persist = ctx.enter_context(tc.tile_pool(name="persist", bufs=1))
        dram = ctx.enter_context(tc.tile_pool(name="dram", bufs=1, space="DRAM"))

        partials = [dram.tile([QG, D], BF16, tag=f"partial{qg}", name=f"partial{qg}")
                    for qg in range(NQG)]
        rs_outs = [dram.tile([256, D], BF16, tag=f"rs{qg}", name=f"rs{qg}")
                   for qg in range(NQG)]
        rs_half = {2 * 3 + h: dram.tile([128, D], BF16, tag=f"rsh{h}", name=f"rsh{h}")
                   for h in range(2)}

        # ---- persistent SBUF tensors -------------------------------------
        qt = [persist.tile([128, S], MM_DT, tag=f"qt{i}", name=f"qt{i}") for i in range(4)]
        kt = [persist.tile([128, S], MM_DT, tag=f"kt{i}", name=f"kt{i}") for i in range(4)]
        vt = [persist.tile([128, HG, DEPTH + 1], MM_DT, tag=f"vt{t}", name=f"vt{t}")
              for t in range(NKB)]
        wq_sb = persist.tile([128, 8, GD], MM_DT, tag="wq_sb")
        wk_sb = persist.tile([128, 8, GD], MM_DT, tag="wk_sb")
        wv_sb = persist.tile([128, 8, GD], MM_DT, tag="wv_sb")
        wosb = persist.tile([128, 4, D], MM_DT, tag="wosb")
        mw = [persist.tile([KB, windows[w].shape[1]], MM_DT, tag=f"mw{w}", name=f"mw{w}")
              for w in range(nwin)]
        bo_bc = persist.tile([128, D], F32, tag="bo_bc")
        bv_bc = persist.tile([128, GD], F32, tag="bv_bc")
        bqt = [persist.tile([128, 1], F32, tag=f"bq{m}", name=f"bq{m}") for m in range(4)]
        ones65 = persist.tile([65, DEPTH], F32R, tag="ones65")
        bkt = [persist.tile([128, 1], F32, tag=f"bk{m}", name=f"bk{m}") for m in range(4)]

        RR2 = [nc.sync, nc.gpsimd]               # in-attention loads
        RRP = [nc.scalar, nc.sync, nc.gpsimd]    # pre-attention loads

        def load_w(wd, dst, rr):
            for kk in range(8):
                rr[kk % len(rr)].dma_start(
                    out=dst[:, kk, :], in_=wd[kk * 128:(kk + 1) * 128, :]
                )

        pps = ctx.enter_context(tc.tile_pool(name="pps", bufs=3, space="PSUM"))
        pacc = ctx.enter_context(tc.tile_pool(name="pacc", bufs=2, space="PSUM"))

        xw_pool = ctx.enter_context(tc.tile_pool(name="xw", bufs=3))
        pt_pool = ctx.enter_context(tc.tile_pool(name="ptp", bufs=8))
        ot_pool = ctx.enter_context(tc.tile_pool(name="otp", bufs=3))
        nrm_pool = ctx.enter_context(tc.tile_pool(name="nrm", bufs=3))
        osb_pool = ctx.enter_context(tc.tile_pool(name="osb", bufs=3))

        xt_cells = {}

        def get_xt(xd, pair, rr):
            key = (id(xd), pair)
            if key not in xt_cells:
                xt = xw_pool.tile([128, 8, 2 * QG], MM_DT, tag="xt")
                for kk in range(8):
                    rr[kk % len(rr)].dma_start(
                        out=xt[:, kk, :],
                        in_=xd[kk * 128:(kk + 1) * 128,
                               pair * 2 * QG:(pair + 1) * 2 * QG],
                    )
                xt_cells[key] = xt
            return xt_cells[key]

        def proj_qk_tg(xd, wt, bias_tiles, dst, tg, rr=None):
            xt = get_xt(xd, tg // 2, rr or RR2)
            h0 = (tg % 2) * QG
            for mh in range(2):
                ps = pps.tile([128, 2 * QG], F32, tag="ps")
                for mm_ in range(2):
                    m = mh * 2 + mm_
                    for kk in range(8):
                        nc.tensor.matmul(
                            ps[:, mm_ * QG:(mm_ + 1) * QG],
                            wt[:, kk, m * 128:(m + 1) * 128],
                            xt[:, kk, h0:h0 + QG],
                            start=(kk == 0),
                            stop=(kk == 7),
                        )
                for mm_ in range(2):
                    m = mh * 2 + mm_
                    nc.vector.tensor_scalar_add(
                        dst[m][:, tg * QG:(tg + 1) * QG],
                        ps[:, mm_ * QG:(mm_ + 1) * QG],
                        bias_tiles[m][:, :],
                    )

        def proj_v_tg(tg, rr=None):
            xt = get_xt(xv, tg // 2, rr or RR2)
            h0 = (tg % 2) * QG
            for th in range(2):
                ps = pps.tile([128, 2 * GD], F32, tag="ps")
                for ts_ in range(2):
                    ts = th * 2 + ts_
                    for kk in range(8):
                        nc.tensor.matmul(
                            ps[:, ts_ * GD:(ts_ + 1) * GD],
                            xt[:, kk, h0 + ts * 128:h0 + (ts + 1) * 128],
                            wv_sb[:, kk, :],
                            start=(kk == 0),
                            stop=(kk == 7),
                        )
                for ts_ in range(2):
                    t = tg * 4 + th * 2 + ts_
                    nc.vector.tensor_tensor(
                        vt[t][:, :, 0:DEPTH],
                        ps[:, ts_ * GD:(ts_ + 1) * GD].rearrange(
                            "p (h d) -> p h d", h=HG),
                        bv_bc[:, :].rearrange("p (h d) -> p h d", h=HG),
                        ALU.add,
                    )

        def attn(qg, ot, fillers=None, slots_per_i=1, pre_av=()):
            """Attention for one query group. fillers: deque of small emit
            units popped (slots_per_i per head-pair) to give the PE work
            while it waits on the exp stream."""
            blocks = sched[qg]
            nb = len(blocks)
            qgc0 = qg * QG
            for i in range(4):  # head pair: heads 2i (rows 0:64), 2i+1 (64:128)
                pts = []
                accs = [pacc.tile([128, QG], F32, tag="acc", name=f"acc{p}")
                        for p in range(2)]

                def do_scores(bj, i=i):
                    kb, c0, w0, w1, wix = blocks[bj]
                    kbc = slice(kb * KB, (kb + 1) * KB)
                    sps = pps.tile([128, 2 * QG], F32, tag="ps")
                    for p, off in ((0, 0), (1, 64)):
                        nc.tensor.matmul(
                            sps[:, p * QG + c0:(p + 1) * QG],
                            kt[i][off:off + 64, kbc],
                            qt[i][off:off + 64, qgc0 + c0:qgc0 + QG],
                            start=True,
                            stop=True,
                        )
                    pt = pt_pool.tile([KB, 2, QG], MM_DT, tag="pt")
                    sps3 = sps[:, :].rearrange("k (t q) -> k t q", t=2)
                    nc.scalar.activation(
                        pt[:, :, c0:QG], sps3[:, :, c0:QG], AF.Exp, scale=0.125
                    )
                    if wix is not None:
                        m_ap = mw[wix][:, :]
                        rep = bass.AP(
                            tensor=m_ap.tensor,
                            offset=m_ap.offset,
                            ap=[list(m_ap.ap[0]), [0, 2], list(m_ap.ap[1])],
                        )
                        nc.vector.tensor_tensor(
                            pt[:, :, w0:w1], pt[:, :, w0:w1], rep, ALU.mult,
                        )
                    pts.append((c0, pt))

                def do_av(bj, i=i):
                    kb = blocks[bj][0]
                    c0, pt = pts[bj]
                    for p in range(2):
                        nc.tensor.matmul(
                            accs[p][0:DEPTH + 1, c0:QG],
                            vt[kb][:, 2 * i + p, :],
                            pt[:, p, c0:QG],
                            start=(bj == 0),
                            stop=(bj == nb - 1),
                        )

                do_scores(0)
                if nb > 1:
                    do_scores(1)
                if i == 0:
                    for u in pre_av:
                        u()
                for bj in range(nb):
                    if bj + 2 < nb:
                        do_scores(bj + 2)
                    do_av(bj)
                # evict AV result to SBUF, then normalize:
                # O^T = o_un[0:64] * (1 / o_un[64])
                for p, acc in enumerate(accs):
                    o_un = nrm_pool.tile([DEPTH + 1, QG], F32, tag="o_un")
                    nc.vector.tensor_copy(o_un[:, :], acc[0:DEPTH + 1, :])
                    rc0 = nrm_pool.tile([1, QG], F32, tag="rc0")
                    nc.sync.dma_start(out=rc0[:, :], in_=o_un[64:65, :])
                    rcr = nrm_pool.tile([1, QG], F32, tag="rcr")
                    nc.vector.reciprocal_approx_fast(rcr[:, :], rc0[:, :])
                    rb = nrm_pool.tile([64, QG], F32, tag="rb")
                    nc.gpsimd.partition_broadcast(rb[:, :], rcr[:, :])
                    if p == 0:
                        nc.vector.tensor_tensor(
                            ot[i][0:64, :], o_un[0:64, :], rb[:, :], ALU.mult
                        )
                    else:
                        tmp = nrm_pool.tile([64, QG], MM_DT, tag="tmp")
                        nc.vector.tensor_tensor(
                            tmp[:, :], o_un[0:64, :], rb[:, :], ALU.mult
                        )
                        nc.gpsimd.dma_start(out=ot[i][64:128, :], in_=tmp[:, :])
                for _ in range(slots_per_i):
                    while fillers:
                        unit = fillers.popleft()
                        unit[0]()
                        if not unit[2]:  # stop after one compute unit
                            break

        def outproj(qg, ot):
            for ts in range(4):
                for nn in range(2):
                    po = pacc.tile([128, QG], F32, tag="acc", name="po")
                    for kk in range(4):
                        nc.tensor.matmul(
                            po[:, :],
                            ot[kk][:, ts * 128:(ts + 1) * 128],
                            wosb[:, kk, nn * QG:(nn + 1) * QG],
                            start=(kk == 0),
                            stop=(kk == 3),
                        )
                    osb = osb_pool.tile([128, QG], MM_DT, tag="osb")
                    nc.vector.tensor_tensor(
                        osb[:, :], po[:, :],
                        bo_bc[:, nn * QG:(nn + 1) * QG], ALU.add,
                    )
                    nc.sync.dma_start(
                        out=partials[qg][ts * 128:(ts + 1) * 128,
                                         nn * QG:(nn + 1) * QG],
                        in_=osb[:, :],
                    )
            nc.gpsimd.collective_compute(
                "ReduceScatter",
                ALU.add,
                replica_groups=GROUPS,
                ins=[partials[qg][:, :]],
                outs=[rs_outs[qg][:, :]],
            )
            nc.sync.dma_start(
                out=out_d[qg * 256:(qg + 1) * 256, :],
                in_=rs_outs[qg][:, :],
            )

        def new_ot():
            return [ot_pool.tile([128, QG], MM_DT, tag=f"ot{i}", name=f"ot{i}")
                    for i in range(4)]

        # ---- filler units (half-projections / half-outproj) --------------
        from collections import deque

        def qk_units(xd, wt, bias_tiles, dst, tg, label):
            def half(mh):
                def emit():
                    xt = get_xt(xd, tg // 2, RR2)
                    h0 = (tg % 2) * QG
                    ps = pps.tile([128, 2 * QG], F32, tag="ps")
                    for mm_ in range(2):
                        m = mh * 2 + mm_
                        for kk in range(8):
                            nc.tensor.matmul(
                                ps[:, mm_ * QG:(mm_ + 1) * QG],
                                wt[:, kk, m * 128:(m + 1) * 128],
                                xt[:, kk, h0:h0 + QG],
                                start=(kk == 0),
                                stop=(kk == 7),
                            )
                    for mm_ in range(2):
                        m = mh * 2 + mm_
                        nc.vector.tensor_scalar_add(
                            dst[m][:, tg * QG:(tg + 1) * QG],
                            ps[:, mm_ * QG:(mm_ + 1) * QG],
                            bias_tiles[m][:, :],
                        )
                return emit
            return [(half(0), label, False), (half(1), label, False)]

        def v_units(tg, label):
            def half(th):
                def emit():
                    xt = get_xt(xv, tg // 2, RR2)
                    h0 = (tg % 2) * QG
                    ps = pps.tile([128, 2 * GD], F32, tag="ps")
                    for ts_ in range(2):
                        ts = th * 2 + ts_
                        for kk in range(8):
                            nc.tensor.matmul(
                                ps[:, ts_ * GD:(ts_ + 1) * GD],
                                xt[:, kk, h0 + ts * 128:h0 + (ts + 1) * 128],
                                wv_sb[:, kk, :],
                                start=(kk == 0),
                                stop=(kk == 7),
                            )
                    for ts_ in range(2):
                        t = tg * 4 + th * 2 + ts_
                        nc.vector.tensor_tensor(
                            vt[t][:, :, 0:DEPTH],
                            ps[:, ts_ * GD:(ts_ + 1) * GD].rearrange(
                                "p (h d) -> p h d", h=HG),
                            bv_bc[:, :].rearrange("p (h d) -> p h d", h=HG),
                            ALU.add,
                        )
                return emit
            return [(half(0), label, False), (half(1), label, False)]

        def xload_unit(xd, pair, label):
            def emit():
                get_xt(xd, pair, RR2)
            return (emit, label, True)

        pending_copy = [None]

        def op_units(qg, ot, label, split_rs=False):
            def fire_rs(r0, nrows, dst_row, rsbuf):
                # the out_d copy of the PREVIOUS ReduceScatter is emitted
                # here, long after that collective finished — emitting it
                # right after its own RS head-of-line blocks the sync queue
                # for the collective's full (peer-synced) duration
                if pending_copy[0] is not None:
                    pending_copy[0]()
                nc.gpsimd.collective_compute(
                    "ReduceScatter",
                    ALU.add,
                    replica_groups=GROUPS,
                    ins=[partials[qg][r0:r0 + nrows, :]],
                    outs=[rsbuf[:, :]],
                )
                pending_copy[0] = lambda: nc.sync.dma_start(
                    out=out_d[dst_row:dst_row + nrows // 2, :],
                    in_=rsbuf[:, :],
                )
            def half(hh):
                def emit():
                    for ts in (2 * hh, 2 * hh + 1):
                        for nn in range(2):
                            po = pacc.tile([128, QG], F32, tag="acc", name="po")
                            for kk in range(4):
                                nc.tensor.matmul(
                                    po[:, :],
                                    ot[kk][:, ts * 128:(ts + 1) * 128],
                                    wosb[:, kk, nn * QG:(nn + 1) * QG],
                                    start=(kk == 0),
                                    stop=(kk == 3),
                                )
                            osb = osb_pool.tile([128, QG], MM_DT, tag="osb")
                            nc.vector.tensor_tensor(
                                osb[:, :], po[:, :],
                                bo_bc[:, nn * QG:(nn + 1) * QG], ALU.add,
                            )
                            nc.sync.dma_start(
                                out=partials[qg][ts * 128:(ts + 1) * 128,
                                                 nn * QG:(nn + 1) * QG],
                                in_=osb[:, :],
                            )
                    if split_rs:
                        fire_rs(hh * 256, 256, qg * 256 + hh * 128,
                                rs_half[2 * qg + hh])
                    elif hh == 1:
                        fire_rs(0, 512, qg * 256, rs_outs[qg])
                return emit
            return [(half(0), label, False), (half(1), label, False)]

        def flush(fillers, label):
            while fillers and any(u[1] == label for u in fillers):
                fillers.popleft()[0]()

        # ---- emission, ordered by need -----------------------------------
        for m in range(4):
            nc.scalar.dma_start(out=bkt[m], in_=bk_d[m * 128:(m + 1) * 128])
        load_w(wk_d, wk_sb, RRP)
        get_xt(xk, 0, RRP)
        proj_qk_tg(xk, wk_sb, bkt, kt, 0, rr=RRP)
        nc.scalar.dma_start(
            out=bv_bc,
            in_=bass.AP(tensor=bv_d, offset=0, ap=[[0, 128], [1, GD]]),
        )
        load_w(wv_d, wv_sb, RRP)
        proj_qk_tg(xk, wk_sb, bkt, kt, 1, rr=RRP)
        for t in range(NKB):
            nc.scalar.dma_start(out=vt[t][:, :, DEPTH:DEPTH + 1], in_=ones_d[:, :])
        nc.scalar.dma_start(out=ones65[64:65, :], in_=onesf_d[:, :].bitcast(F32R))
        get_xt(xv, 0, RRP)
        for m in range(4):
            nc.scalar.dma_start(out=bqt[m], in_=bq_d[m * 128:(m + 1) * 128])
        load_w(wq_d, wq_sb, RRP)
        get_xt(xq, 0, RRP)
        proj_qk_tg(xq, wq_sb, bqt, qt, 1, rr=RRP)
        for w in range(nwin):
            nc.scalar.dma_start(out=mw[w], in_=win_d[w][:, :])
        for kk in range(4):
            nc.sync.dma_start(
                out=wosb[:, kk, :], in_=wo_d[kk * 128:(kk + 1) * 128, :]
            )
        nc.scalar.dma_start(
            out=bo_bc,
            in_=bass.AP(tensor=bo_d, offset=0, ap=[[0, 128], [1, D]]),
        )

        ot1 = new_ot()
        ot0 = new_ot()
        uq0 = qk_units(xq, wq_sb, bqt, qt, 0, "q0")
        uk2 = qk_units(xk, wk_sb, bkt, kt, 2, "k2")
        uv2 = v_units(2, "v2")
        uq2 = qk_units(xq, wq_sb, bqt, qt, 2, "q2")
        uop1 = op_units(1, ot1, "op1")
        uk3 = qk_units(xk, wk_sb, bkt, kt, 3, "k3")
        uv3 = v_units(3, "v3")
        uq3 = qk_units(xq, wq_sb, bqt, qt, 3, "q3")
        uop0 = op_units(0, ot0, "op0")
        fill = deque([
            uq0[0], uq0[1], xload_unit(xk, 1, "k2"),
            uk2[0], uk2[1], xload_unit(xv, 1, "v2"),
            uv2[0], uv2[1], xload_unit(xq, 1, "q2"),
            uq2[0], uq2[1],
            uop1[0], uop1[1],
            uk3[0], uk3[1],
            uv3[0], uv3[1],
            uq3[0], uq3[1],
            uop0[0], uop0[1],
        ])

        pv = v_units(0, "pv") + v_units(1, "pv")
        attn(1, ot1, fillers=fill, slots_per_i=1,
             pre_av=[u[0] for u in pv])
        flush(fill, "q0")
        attn(0, ot0, fillers=fill, slots_per_i=2)
        flush(fill, "k2")
        flush(fill, "v2")
        flush(fill, "q2")
        ot2 = new_ot()
        attn(2, ot2, fillers=fill, slots_per_i=2)
        flush(fill, "op1")
        flush(fill, "k3")
        flush(fill, "v3")
        flush(fill, "q3")
        ot3 = new_ot()
        fill.extend(op_units(2, ot2, "op2"))
        attn(3, ot3, fillers=fill, slots_per_i=2)
        while fill:
            fill.popleft()[0]()
        for u in op_units(3, ot3, "op3", split_rs=True):
            u[0]()
        if pending_copy[0] is not None:
            pending_copy[0]()

    nc.finalize()
    return nc


_CACHED = {}


def _get_nc(mask2d):
    key = mask2d.tobytes()
    if key not in _CACHED:
        _CACHED[key] = _build(*_mask_schedule(mask2d))
    return _CACHED[key]


def kernel(v, k, q, mask, wq, bq, wk, bk, wv, bv, wo, bo, _trace=False):
    global LAST_EXEC_NS, LAST_RESULTS
    f = lambda a: np.asarray(a, dtype=np.float32)
    v, k, q = f(v), f(k), f(q)
    wq, wk, wv, wo = f(wq), f(wk), f(wv), f(wo)
    bq, bk, bv, bo = f(bq), f(bk), f(bv), f(bo)
    mask2d = f(mask).reshape(S, S)

    nc = _get_nc(mask2d)

    bf = lambda a: np.ascontiguousarray(a).astype(ml_dtypes.bfloat16)
    in_maps = []
    for c in range(NCORES):
        b, g = c // 2, c % 2
        cols = slice(g * GD, (g + 1) * GD)
        in_maps.append({
            "xq_t": bf(q[b].T),
            "xk_t": bf(k[b].T),
            "xv_t": bf(v[b].T),
            "wq_g": bf(wq[:, cols]),
            "wk_g": bf(wk[:, cols]),
            "wv_g": bf(wv[:, cols]),
            "wo_g": bf(wo[cols, :]),
            "bq_g": np.ascontiguousarray(bq[cols]),
            "bk_g": np.ascontiguousarray(bk[cols]),
            "bv_g": np.ascontiguousarray(bv[cols]),
            "bo_h": np.ascontiguousarray(bo * np.float32(0.5)),
        })

    res = run_bass_kernel_spmd(
        nc, in_maps, core_ids=list(range(NCORES)), trace=_trace
    )
    LAST_EXEC_NS = res.exec_time_ns
    LAST_RESULTS = res

    out = np.empty((B, S, D), dtype=np.float32)
    for c in range(NCORES):
        b, g = c // 2, c % 2
        o = np.asarray(res.results[c]["out"], dtype=np.float32)  # [1024, D]
        for qg in range(NQG):
            if qg == 3:  # split ReduceScatter: two 128-row chunks
                for h in range(2):
                    gr = qg * QG + h * 256 + g * 128
                    out[b, gr:gr + 128, :] = o[qg * 256 + h * 128:
                                               qg * 256 + (h + 1) * 128, :]
            else:
                gr = qg * QG + g * 256
                out[b, gr:gr + 256, :] = o[qg * 256:(qg + 1) * 256, :]
    return out


# revision 47
# speedup vs baseline: 1.0106x; 1.0106x over previous
"""Multi-head self-attention (B=4, S=2048, D=1024, H=16) on 8 TRN2 NeuronCores.

Sharding: core c handles batch b = c // 2 and head-group g = c % 2
(8 heads, 512 hidden columns). Per core:
  - Q^T, K^T projections (d-major layout), V projection (token-major),
    emitted per 512-token group, need-ordered so the first score matmul
    fires a few microseconds in; later token groups are interleaved into
    the attention stream as PE filler (the scalar engine's exp stream is
    the global bottleneck).
  - Attention computed transposed: S^T[k, q] = K_h @ Q_h^T per 128-key
    block, two heads concurrently on disjoint PE row-groups; exp on the
    scalar engine (softmax max-subtraction is skipped — logits ~N(0,1)).
    Fully-masked query columns of partially-masked blocks are skipped in
    the scores matmul, exp, and AV accumulation; the remaining partial
    window is multiplied by a (1-mask) triangle on the vector engine.
  - O^T = V_aug^T @ P^T with a ones column appended to V so the softmax
    denominator drops out of the same matmul; normalize by its
    reciprocal.
  - Row-sharded output projection -> partial [512, 1024] bf16 per query
    group, one pairwise bf16 ReduceScatter per query group (the last
    group fires two half-sized ones so the collective overlaps its own
    projection), final rows DMA'd to the output.
Host reassembles: rows qg*512 + g*256 of batch b come from core (2*b+g);
the last query group is mapped per 128-row half-chunk.
"""

import sys

for _p in ("/opt/trn_rl_repo",):
    if _p not in sys.path:
        sys.path.insert(0, _p)

from contextlib import ExitStack

import ml_dtypes
import numpy as np

import concourse.bass as bass
from concourse import bacc
import concourse.mybir as mybir
import concourse.tile as tile
from concourse.bass_utils import run_bass_kernel_spmd

F32 = mybir.dt.float32
F32R = mybir.dt.float32r
BF16 = mybir.dt.bfloat16
MM_DT = BF16
AF = mybir.ActivationFunctionType
ALU = mybir.AluOpType

B, S, D, H, DEPTH = 4, 2048, 1024, 16, 64
HG = H // 2          # heads per core = 8
GD = HG * DEPTH      # local hidden width = 512
QG = 512             # query-group width (matmul N)
KB = 128             # key-block height (matmul M)
NQG = S // QG        # 4
NKB = S // KB        # 16
NCORES = 8

LAST_EXEC_NS = None
LAST_RESULTS = None

QG_ORDER = (1, 2, 3, 0)
GROUPS = [[0, 1], [2, 3], [4, 5], [6, 7]]


def _mask_schedule(mask2d):
    """Classify each (query-group, key-block) against the actual mask.

    Returns (sched, windows): sched[qg] is a list of (kb, c0, w0, w1, widx).
    Query columns [0, c0) of the block are fully masked and skipped
    entirely; columns [w0, w1) are partially masked and get multiplied by
    windows[widx] ([128, w1-w0], value 1-mask transposed to [k, q]);
    columns [w1, 512) are fully allowed. A block is dropped when fully
    masked. The first surviving block of each qg is forced to c0=0 so its
    AV/scores matmuls initialize the whole PSUM accumulation range.
    """
    wins = {}
    warr = []
    sched = []
    for qg in range(NQG):
        blocks = []
        for kb in range(NKB):
            blk = mask2d[qg * QG:(qg + 1) * QG, kb * KB:(kb + 1) * KB]  # [q, k]
            assert set(np.unique(blk)) <= {0.0, 1.0}, "non-binary mask"
            col_masked = blk.all(axis=1)      # [q] fully masked query?
            col_any = blk.any(axis=1)         # [q] any masked key?
            if col_masked.all():
                continue  # fully masked block
            c0 = 0
            while c0 < QG and col_masked[c0]:
                c0 += 1
            if not blocks:
                c0 = 0  # first block initializes full PSUM range
            w1 = QG
            while w1 > c0 and not col_any[w1 - 1]:
                w1 -= 1
            w0 = c0
            if w1 <= w0:
                blocks.append((kb, c0, 0, 0, None))
                continue
            w = (1.0 - np.ascontiguousarray(blk[w0:w1, :].T)).astype(np.float32)
            key = w.tobytes()
            if key not in wins:
                wins[key] = len(warr)
                warr.append(w)
            blocks.append((kb, c0, w0, w1, wins[key]))
        sched.append(blocks)
    assert len(warr) <= 16, "too many unique mask windows"
    return sched, warr


def _build(sched, windows):
    nc = bacc.Bacc(target_bir_lowering=False, trn_type="TRN2")

    xq = nc.dram_tensor("xq_t", [D, S], BF16, kind="ExternalInput")
    xk = nc.dram_tensor("xk_t", [D, S], BF16, kind="ExternalInput")
    xv = nc.dram_tensor("xv_t", [D, S], BF16, kind="ExternalInput")
    wq_d = nc.dram_tensor("wq_g", [D, GD], BF16, kind="ExternalInput")
    wk_d = nc.dram_tensor("wk_g", [D, GD], BF16, kind="ExternalInput")
    wv_d = nc.dram_tensor("wv_g", [D, GD], BF16, kind="ExternalInput")
    wo_d = nc.dram_tensor("wo_g", [GD, D], BF16, kind="ExternalInput")
    bq_d = nc.dram_tensor("bq_g", [GD], F32, kind="ExternalInput")
    bk_d = nc.dram_tensor("bk_g", [GD], F32, kind="ExternalInput")
    bv_d = nc.dram_tensor("bv_g", [GD], F32, kind="ExternalInput")
    bo_d = nc.dram_tensor("bo_h", [D], F32, kind="ExternalInput")
    out_d = nc.dram_tensor("out", [NQG * 256, D], BF16, kind="ExternalOutput")

    ones_d = nc.inline_tensor(np.ones((128, HG), dtype=ml_dtypes.bfloat16), "ones_col")
    onesf_d = nc.inline_tensor(np.ones((1, DEPTH), dtype=np.float32), "ones_row")
    nwin = len(windows)
    win_d = [
        nc.inline_tensor(w.astype(ml_dtypes.bfloat16), f"mwin{i}")
        for i, w in enumerate(windows)
    ]

    with tile.TileContext(nc) as tc, ExitStack() as ctx:
        persist = ctx.enter_context(tc.tile_pool(name="persist", bufs=1))
        dram = ctx.enter_context(tc.tile_pool(name="dram", bufs=1, space="DRAM"))

        partials = [dram.tile([QG, D], BF16, tag=f"partial{qg}", name=f"partial{qg}")
                    for qg in range(NQG)]
        rs_outs = [dram.tile([256, D], BF16, tag=f"rs{qg}", name=f"rs{qg}")
                   for qg in range(NQG)]
        rs_half = {2 * 3 + h: dram.tile([128, D], BF16, tag=f"rsh{h}", name=f"rsh{h}")
                   for h in range(2)}

        # ---- persistent SBUF tensors -------------------------------------
        qt = [persist.tile([128, S], MM_DT, tag=f"qt{i}", name=f"qt{i}") for i in range(4)]
        kt = [persist.tile([128, S], MM_DT, tag=f"kt{i}", name=f"kt{i}") for i in range(4)]
        vt = [persist.tile([128, HG, DEPTH + 1], MM_DT, tag=f"vt{t}", name=f"vt{t}")
              for t in range(NKB)]
        wq_sb = persist.tile([128, 8, GD], MM_DT, tag="wq_sb")
        wk_sb = persist.tile([128, 8, GD], MM_DT, tag="wk_sb")
        wv_sb = persist.tile([128, 8, GD], MM_DT, tag="wv_sb")
        wosb = persist.tile([128, 4, D], MM_DT, tag="wosb")
        mw = [persist.tile([KB, windows[w].shape[1]], MM_DT, tag=f"mw{w}", name=f"mw{w}")
              for w in range(nwin)]
        bo_bc = persist.tile([128, D], F32, tag="bo_bc")
        bv_bc = persist.tile([128, GD], F32, tag="bv_bc")
        bqt = [persist.tile([128, 1], F32, tag=f"bq{m}", name=f"bq{m}") for m in range(4)]
        ones65 = persist.tile([65, DEPTH], F32R, tag="ones65")
        bkt = [persist.tile([128, 1], F32, tag=f"bk{m}", name=f"bk{m}") for m in range(4)]

        RR2 = [nc.sync, nc.gpsimd]               # in-attention loads
        RRP = [nc.scalar, nc.sync, nc.gpsimd]    # pre-attention loads

        def load_w(wd, dst, rr):
            for kk in range(8):
                rr[kk % len(rr)].dma_start(
                    out=dst[:, kk, :], in_=wd[kk * 128:(kk + 1) * 128, :]
                )

        pps = ctx.enter_context(tc.tile_pool(name="pps", bufs=3, space="PSUM"))
        pacc = ctx.enter_context(tc.tile_pool(name="pacc", bufs=2, space="PSUM"))

        xw_pool = ctx.enter_context(tc.tile_pool(name="xw", bufs=3))
        pt_pool = ctx.enter_context(tc.tile_pool(name="ptp", bufs=8))
        ot_pool = ctx.enter_context(tc.tile_pool(name="otp", bufs=3))
        nrm_pool = ctx.enter_context(tc.tile_pool(name="nrm", bufs=3))
        osb_pool = ctx.enter_context(tc.tile_pool(name="osb", bufs=3))

        xt_cells = {}

        def get_xt(xd, pair, rr):
            key = (id(xd), pair)
            if key not in xt_cells:
                xt = xw_pool.tile([128, 8, 2 * QG], MM_DT, tag="xt")
                for kk in range(8):
                    rr[kk % len(rr)].dma_start(
                        out=xt[:, kk, :],
                        in_=xd[kk * 128:(kk + 1) * 128,
                               pair * 2 * QG:(pair + 1) * 2 * QG],
                    )
                xt_cells[key] = xt
            return xt_cells[key]

        def proj_qk_tg(xd, wt, bias_tiles, dst, tg, rr=None):
            xt = get_xt(xd, tg // 2, rr or RR2)
            h0 = (tg % 2) * QG
            for mh in range(2):
                ps = pps.tile([128, 2 * QG], F32, tag="ps")
                for mm_ in range(2):
                    m = mh * 2 + mm_
                    for kk in range(8):
                        nc.tensor.matmul(
                            ps[:, mm_ * QG:(mm_ + 1) * QG],
                            wt[:, kk, m * 128:(m + 1) * 128],
                            xt[:, kk, h0:h0 + QG],
                            start=(kk == 0),
                            stop=(kk == 7),
                        )
                for mm_ in range(2):
                    m = mh * 2 + mm_
                    nc.vector.tensor_scalar_add(
                        dst[m][:, tg * QG:(tg + 1) * QG],
                        ps[:, mm_ * QG:(mm_ + 1) * QG],
                        bias_tiles[m][:, :],
                    )

        def proj_v_tg(tg, rr=None):
            xt = get_xt(xv, tg // 2, rr or RR2)
            h0 = (tg % 2) * QG
            for th in range(2):
                ps = pps.tile([128, 2 * GD], F32, tag="ps")
                for ts_ in range(2):
                    ts = th * 2 + ts_
                    for kk in range(8):
                        nc.tensor.matmul(
                            ps[:, ts_ * GD:(ts_ + 1) * GD],
                            xt[:, kk, h0 + ts * 128:h0 + (ts + 1) * 128],
                            wv_sb[:, kk, :],
                            start=(kk == 0),
                            stop=(kk == 7),
                        )
                for ts_ in range(2):
                    t = tg * 4 + th * 2 + ts_
                    nc.vector.tensor_tensor(
                        vt[t][:, :, 0:DEPTH],
                        ps[:, ts_ * GD:(ts_ + 1) * GD].rearrange(
                            "p (h d) -> p h d", h=HG),
                        bv_bc[:, :].rearrange("p (h d) -> p h d", h=HG),
                        ALU.add,
                    )

        def attn(qg, ot, fillers=None, slots_per_i=1, pre_av=()):
            """Attention for one query group. fillers: deque of small emit
            units popped (slots_per_i per head-pair) to give the PE work
            while it waits on the exp stream."""
            blocks = sched[qg]
            nb = len(blocks)
            qgc0 = qg * QG
            for i in range(4):  # head pair: heads 2i (rows 0:64), 2i+1 (64:128)
                pts = []
                accs = [pacc.tile([128, QG], F32, tag="acc", name=f"acc{p}")
                        for p in range(2)]

                def do_scores(bj, i=i):
                    kb, c0, w0, w1, wix = blocks[bj]
                    kbc = slice(kb * KB, (kb + 1) * KB)
                    sps = pps.tile([128, 2 * QG], F32, tag="ps")
                    for p, off in ((0, 0), (1, 64)):
                        nc.tensor.matmul(
                            sps[:, p * QG + c0:(p + 1) * QG],
                            kt[i][off:off + 64, kbc],
                            qt[i][off:off + 64, qgc0 + c0:qgc0 + QG],
                            start=True,
                            stop=True,
                        )
                    pt = pt_pool.tile([KB, 2, QG], MM_DT, tag="pt")
                    sps3 = sps[:, :].rearrange("k (t q) -> k t q", t=2)
                    nc.scalar.activation(
                        pt[:, :, c0:QG], sps3[:, :, c0:QG], AF.Exp, scale=0.125
                    )
                    if wix is not None:
                        m_ap = mw[wix][:, :]
                        rep = bass.AP(
                            tensor=m_ap.tensor,
                            offset=m_ap.offset,
                            ap=[list(m_ap.ap[0]), [0, 2], list(m_ap.ap[1])],
                        )
                        nc.vector.tensor_tensor(
                            pt[:, :, w0:w1], pt[:, :, w0:w1], rep, ALU.mult,
                        )
                    pts.append((c0, pt))

                def do_av(bj, i=i):
                    kb = blocks[bj][0]
                    c0, pt = pts[bj]
                    for p in range(2):
                        nc.tensor.matmul(
                            accs[p][0:DEPTH + 1, c0:QG],
                            vt[kb][:, 2 * i + p, :],
                            pt[:, p, c0:QG],
                            start=(bj == 0),
                            stop=(bj == nb - 1),
                        )

                do_scores(0)
                if nb > 1:
                    do_scores(1)
                if i == 0:
                    for u in pre_av:
                        u()
                for bj in range(nb):
                    if bj + 2 < nb:
                        do_scores(bj + 2)
                    do_av(bj)
                # evict AV result to SBUF, then normalize:
                # O^T = o_un[0:64] * (1 / o_un[64])
                for p, acc in enumerate(accs):
                    o_un = nrm_pool.tile([DEPTH + 1, QG], F32, tag="o_un")
                    nc.vector.tensor_copy(o_un[:, :], acc[0:DEPTH + 1, :])
                    rc0 = nrm_pool.tile([1, QG], F32, tag="rc0")
                    nc.sync.dma_start(out=rc0[:, :], in_=o_un[64:65, :])
                    rcr = nrm_pool.tile([1, QG], F32, tag="rcr")
                    nc.vector.reciprocal_approx_fast(rcr[:, :], rc0[:, :])
                    rb = nrm_pool.tile([64, QG], F32, tag="rb")
                    nc.gpsimd.partition_broadcast(rb[:, :], rcr[:, :])
                    if p == 0:
                        nc.vector.tensor_tensor(
                            ot[i][0:64, :], o_un[0:64, :], rb[:, :], ALU.mult
                        )
                    else:
                        tmp = nrm_pool.tile([64, QG], MM_DT, tag="tmp")
                        nc.vector.tensor_tensor(
                            tmp[:, :], o_un[0:64, :], rb[:, :], ALU.mult
                        )
                        nc.gpsimd.dma_start(out=ot[i][64:128, :], in_=tmp[:, :])
                for _ in range(slots_per_i):
                    while fillers:
                        unit = fillers.popleft()
                        unit[0]()
                        if not unit[2]:  # stop after one compute unit
                            break

        def outproj(qg, ot):
            for ts in range(4):
                for nn in range(2):
                    po = pacc.tile([128, QG], F32, tag="acc", name="po")
                    for kk in range(4):
                        nc.tensor.matmul(
                            po[:, :],
                            ot[kk][:, ts * 128:(ts + 1) * 128],
                            wosb[:, kk, nn * QG:(nn + 1) * QG],
                            start=(kk == 0),
                            stop=(kk == 3),
                        )
                    osb = osb_pool.tile([128, QG], MM_DT, tag="osb")
                    nc.vector.tensor_tensor(
                        osb[:, :], po[:, :],
                        bo_bc[:, nn * QG:(nn + 1) * QG], ALU.add,
                    )
                    nc.sync.dma_start(
                        out=partials[qg][ts * 128:(ts + 1) * 128,
                                         nn * QG:(nn + 1) * QG],
                        in_=osb[:, :],
                    )
            nc.gpsimd.collective_compute(
                "ReduceScatter",
                ALU.add,
                replica_groups=GROUPS,
                ins=[partials[qg][:, :]],
                outs=[rs_outs[qg][:, :]],
            )
            nc.sync.dma_start(
                out=out_d[qg * 256:(qg + 1) * 256, :],
                in_=rs_outs[qg][:, :],
            )

        def new_ot():
            return [ot_pool.tile([128, QG], MM_DT, tag=f"ot{i}", name=f"ot{i}")
                    for i in range(4)]

        # ---- filler units (half-projections / half-outproj) --------------
        from collections import deque

        def qk_units(xd, wt, bias_tiles, dst, tg, label):
            def half(mh):
                def emit():
                    xt = get_xt(xd, tg // 2, RR2)
                    h0 = (tg % 2) * QG
                    ps = pps.tile([128, 2 * QG], F32, tag="ps")
                    for mm_ in range(2):
                        m = mh * 2 + mm_
                        for kk in range(8):
                            nc.tensor.matmul(
                                ps[:, mm_ * QG:(mm_ + 1) * QG],
                                wt[:, kk, m * 128:(m + 1) * 128],
                                xt[:, kk, h0:h0 + QG],
                                start=(kk == 0),
                                stop=(kk == 7),
                            )
                    for mm_ in range(2):
                        m = mh * 2 + mm_
                        nc.vector.tensor_scalar_add(
                            dst[m][:, tg * QG:(tg + 1) * QG],
                            ps[:, mm_ * QG:(mm_ + 1) * QG],
                            bias_tiles[m][:, :],
                        )
                return emit
            return [(half(0), label, False), (half(1), label, False)]

        def v_units(tg, label):
            def half(th):
                def emit():
                    xt = get_xt(xv, tg // 2, RR2)
                    h0 = (tg % 2) * QG
                    ps = pps.tile([128, 2 * GD], F32, tag="ps")
                    for ts_ in range(2):
                        ts = th * 2 + ts_
                        for kk in range(8):
                            nc.tensor.matmul(
                                ps[:, ts_ * GD:(ts_ + 1) * GD],
                                xt[:, kk, h0 + ts * 128:h0 + (ts + 1) * 128],
                                wv_sb[:, kk, :],
                                start=(kk == 0),
                                stop=(kk == 7),
                            )
                    for ts_ in range(2):
                        t = tg * 4 + th * 2 + ts_
                        nc.vector.tensor_tensor(
                            vt[t][:, :, 0:DEPTH],
                            ps[:, ts_ * GD:(ts_ + 1) * GD].rearrange(
                                "p (h d) -> p h d", h=HG),
                            bv_bc[:, :].rearrange("p (h d) -> p h d", h=HG),
                            ALU.add,
                        )
                return emit
            return [(half(0), label, False), (half(1), label, False)]

        def xload_unit(xd, pair, label):
            def emit():
                get_xt(xd, pair, RR2)
            return (emit, label, True)

        pending_copy = [None]

        def op_units(qg, ot, label, split_rs=False):
            def fire_rs(r0, nrows, dst_row, rsbuf):
                # the out_d copy of the PREVIOUS ReduceScatter is emitted
                # here, long after that collective finished — emitting it
                # right after its own RS head-of-line blocks the sync queue
                # for the collective's full (peer-synced) duration
                if pending_copy[0] is not None:
                    pending_copy[0]()
                nc.gpsimd.collective_compute(
                    "ReduceScatter",
                    ALU.add,
                    replica_groups=GROUPS,
                    ins=[partials[qg][r0:r0 + nrows, :]],
                    outs=[rsbuf[:, :]],
                )
                pending_copy[0] = lambda: nc.sync.dma_start(
                    out=out_d[dst_row:dst_row + nrows // 2, :],
                    in_=rsbuf[:, :],
                )
            def half(hh):
                def emit():
                    for ts in (2 * hh, 2 * hh + 1):
                        for nn in range(2):
                            po = pacc.tile([128, QG], F32, tag="acc", name="po")
                            for kk in range(4):
                                nc.tensor.matmul(
                                    po[:, :],
                                    ot[kk][:, ts * 128:(ts + 1) * 128],
                                    wosb[:, kk, nn * QG:(nn + 1) * QG],
                                    start=(kk == 0),
                                    stop=(kk == 3),
                                )
                            osb = osb_pool.tile([128, QG], MM_DT, tag="osb")
                            nc.vector.tensor_tensor(
                                osb[:, :], po[:, :],
                                bo_bc[:, nn * QG:(nn + 1) * QG], ALU.add,
                            )
                            nc.sync.dma_start(
                                out=partials[qg][ts * 128:(ts + 1) * 128,
                                                 nn * QG:(nn + 1) * QG],
                                in_=osb[:, :],
                            )
                    if split_rs:
                        fire_rs(hh * 256, 256, qg * 256 + hh * 128,
                                rs_half[2 * qg + hh])
                    elif hh == 1:
                        fire_rs(0, 512, qg * 256, rs_outs[qg])
                return emit
            return [(half(0), label, False), (half(1), label, False)]

        def flush(fillers, label):
            while fillers and any(u[1] == label for u in fillers):
                fillers.popleft()[0]()

        # ---- emission, ordered by need -----------------------------------
        for m in range(4):
            nc.gpsimd.dma_start(out=bkt[m], in_=bk_d[m * 128:(m + 1) * 128])
        load_w(wk_d, wk_sb, RRP)
        get_xt(xk, 0, RRP)
        proj_qk_tg(xk, wk_sb, bkt, kt, 0, rr=RRP)
        nc.gpsimd.dma_start(
            out=bv_bc,
            in_=bass.AP(tensor=bv_d, offset=0, ap=[[0, 128], [1, GD]]),
        )
        load_w(wv_d, wv_sb, RRP)
        proj_qk_tg(xk, wk_sb, bkt, kt, 1, rr=RRP)
        for t in range(NKB):
            nc.gpsimd.dma_start(out=vt[t][:, :, DEPTH:DEPTH + 1], in_=ones_d[:, :])
        nc.gpsimd.dma_start(out=ones65[64:65, :], in_=onesf_d[:, :].bitcast(F32R))
        get_xt(xv, 0, RRP)
        for m in range(4):
            nc.gpsimd.dma_start(out=bqt[m], in_=bq_d[m * 128:(m + 1) * 128])
        load_w(wq_d, wq_sb, RRP)
        get_xt(xq, 0, RRP)
        proj_qk_tg(xq, wq_sb, bqt, qt, 1, rr=RRP)
        for w in range(nwin):
            nc.gpsimd.dma_start(out=mw[w], in_=win_d[w][:, :])
        for kk in range(4):
            nc.sync.dma_start(
                out=wosb[:, kk, :], in_=wo_d[kk * 128:(kk + 1) * 128, :]
            )
        nc.gpsimd.dma_start(
            out=bo_bc,
            in_=bass.AP(tensor=bo_d, offset=0, ap=[[0, 128], [1, D]]),
        )

        ot1 = new_ot()
        ot0 = new_ot()
        uq0 = qk_units(xq, wq_sb, bqt, qt, 0, "q0")
        uk2 = qk_units(xk, wk_sb, bkt, kt, 2, "k2")
        uv2 = v_units(2, "v2")
        uq2 = qk_units(xq, wq_sb, bqt, qt, 2, "q2")
        uop1 = op_units(1, ot1, "op1")
        uk3 = qk_units(xk, wk_sb, bkt, kt, 3, "k3")
        uv3 = v_units(3, "v3")
        uq3 = qk_units(xq, wq_sb, bqt, qt, 3, "q3")
        uop0 = op_units(0, ot0, "op0")
        fill = deque([
            uq0[0], uq0[1], xload_unit(xk, 1, "k2"),
            uk2[0], uk2[1], xload_unit(xv, 1, "v2"),
            uv2[0], uv2[1], xload_unit(xq, 1, "q2"),
            uq2[0], uq2[1],
            uop1[0], uop1[1],
            uk3[0], uk3[1],
            uv3[0], uv3[1],
            uq3[0], uq3[1],
            uop0[0], uop0[1],
        ])

        pv = v_units(0, "pv") + v_units(1, "pv")
        attn(1, ot1, fillers=fill, slots_per_i=1,
             pre_av=[u[0] for u in pv])
        flush(fill, "q0")
        attn(0, ot0, fillers=fill, slots_per_i=2)
        flush(fill, "k2")
        flush(fill, "v2")
        flush(fill, "q2")
        ot2 = new_ot()
        attn(2, ot2, fillers=fill, slots_per_i=2)
        flush(fill, "op1")
        flush(fill, "k3")
        flush(fill, "v3")
        flush(fill, "q3")
        ot3 = new_ot()
        fill.extend(op_units(2, ot2, "op2"))
        attn(3, ot3, fillers=fill, slots_per_i=2)
        while fill:
            fill.popleft()[0]()
        for u in op_units(3, ot3, "op3", split_rs=True):
            u[0]()
        if pending_copy[0] is not None:
            pending_copy[0]()

    nc.finalize()
    return nc


_CACHED = {}


def _get_nc(mask2d):
    key = mask2d.tobytes()
    if key not in _CACHED:
        _CACHED[key] = _build(*_mask_schedule(mask2d))
    return _CACHED[key]


def kernel(v, k, q, mask, wq, bq, wk, bk, wv, bv, wo, bo, _trace=False):
    global LAST_EXEC_NS, LAST_RESULTS
    f = lambda a: np.asarray(a, dtype=np.float32)
    v, k, q = f(v), f(k), f(q)
    wq, wk, wv, wo = f(wq), f(wk), f(wv), f(wo)
    bq, bk, bv, bo = f(bq), f(bk), f(bv), f(bo)
    mask2d = f(mask).reshape(S, S)

    nc = _get_nc(mask2d)

    bf = lambda a: np.ascontiguousarray(a).astype(ml_dtypes.bfloat16)
    in_maps = []
    for c in range(NCORES):
        b, g = c // 2, c % 2
        cols = slice(g * GD, (g + 1) * GD)
        in_maps.append({
            "xq_t": bf(q[b].T),
            "xk_t": bf(k[b].T),
            "xv_t": bf(v[b].T),
            "wq_g": bf(wq[:, cols]),
            "wk_g": bf(wk[:, cols]),
            "wv_g": bf(wv[:, cols]),
            "wo_g": bf(wo[cols, :]),
            "bq_g": np.ascontiguousarray(bq[cols]),
            "bk_g": np.ascontiguousarray(bk[cols]),
            "bv_g": np.ascontiguousarray(bv[cols]),
            "bo_h": np.ascontiguousarray(bo * np.float32(0.5)),
        })

    res = run_bass_kernel_spmd(
        nc, in_maps, core_ids=list(range(NCORES)), trace=_trace
    )
    LAST_EXEC_NS = res.exec_time_ns
    LAST_RESULTS = res

    out = np.empty((B, S, D), dtype=np.float32)
    for c in range(NCORES):
        b, g = c // 2, c % 2
        o = np.asarray(res.results[c]["out"], dtype=np.float32)  # [1024, D]
        for qg in range(NQG):
            if qg == 3:  # split ReduceScatter: two 128-row chunks
                for h in range(2):
                    gr = qg * QG + h * 256 + g * 128
                    out[b, gr:gr + 128, :] = o[qg * 256 + h * 128:
                                               qg * 256 + (h + 1) * 128, :]
            else:
                gr = qg * QG + g * 256
                out[b, gr:gr + 256, :] = o[qg * 256:(qg + 1) * 256, :]
    return out
